# revision 37
# baseline (speedup 1.0000x reference)
"""Trainium2 Bass kernel for DetCenterDense: shared 3x3 conv + ReLU + four
1x1 head convs (cls/box/dir/scr, sigmoid on scr), channel-concatenated output.

Full inputs in / full output out. Sharding: 8 cores = batch(4) x H-halves(2);
each core computes a [20, 256, 512] output shard from a [128, 258, 512]
haloed input shard. No inter-core communication.

Compute structure (per core): the 3x3 conv is 9 shifted 1x1 convs accumulated
in PSUM, processing output rows in pairs packed into one PSUM tile [128, 512]
(partitions 0:64 = row y, 64:128 = row y+1). Every matmul in the kernel is an
M=64 column-tile so the PE array stays in (128,64) mode throughout (no
mode-switch drains) and independent M=64 matmuls run two-per-slot:

  - "full" taps (middle input rows, packed [W_a|W_b] weights) issue as two
    concurrent col-tiled twins sharing one moving stream;
  - "edge" taps (rows a/d, M=64 each) pair up in one slot;
  - two consecutive pairs' head matmuls (M=40) share one slot.

That is 9.5 tensor-engine slots per row pair -- the matmul-count floor for
K=128/M<=128 -- and the measured steady state is ~216 ns/slot (N=512 bf16
issue-gap at 2.4 GHz) with <0.2% stall.

Activations stream as fp8e3 (E3M4; x pre-scaled by 2 with 1/2 folded into the
bf16 conv weights) to halve SBUF read traffic -- without this, concurrent
DVE/ACT/DMA traffic degrades the matmul issue rate ~20%. Conv weights stay
bf16 (fp8 weights would breach the 2e-2 accuracy gate; rel l2 is ~1.35e-2).
ReLU runs on DVE; head bias on DVE (identity rows) and ACT (sigmoid rows).

Prologue/epilogue engineering (the stream itself is at the slot floor):
  - all conv/head weights ship as one bf16 tensor whose columns are laid
    out in first-use order and loaded as four descriptors split across the
    scalar and gpsimd queues, each landing just before its taps run; group
    0 consumes taps in descriptor-arrival order (kx1 fulls, kx0 fulls,
    halves, kx2) over a three-piece leading x chunk;
  - a burst of dependency-free N=64 dummy matmuls at program start keeps
    the PE busy through the weight/input DMA wait, releasing the HAM clock
    gate (K=4/8 -> 8/8) before the first real matmul;
  - head outputs are written channel-row-interleaved (partition 2q+e = head
    channel q of row-parity e, scr at 32+2s+e) so each row pair's whole
    [20ch x 2rows x 512] output shard flushes with ONE DMA descriptor,
    issued immediately per pair on a rotating queue -- no end-of-kernel
    descriptor backlog; gpsimd sits out the last 12 pairs so its ~4us
    SWDGE teardown drain overlaps the matmul stream.
"""

import numpy as np

HS = 256          # output rows per core shard
HALO = HS + 2     # input rows per core shard (1-row halo each side)
W = 512
CH = 8            # input rows per DMA chunk
NCHUNK = (HALO + CH - 1) // CH
XSCALE = 2.0      # fp8e3 pre-scale of x; 1/XSCALE folded into w_shared
WARM_MM = 66      # dummy matmuls to release the HAM clock gate at startup

# all conv/head weights ship as ONE bf16 dram tensor, column layout in
# first-use order so per-group DMA descriptors stream in just ahead of the
# matmuls that need them (group 0 runs fulls kx1, fulls kx0, halves, kx2):
#   [kx1 fulls 256][kx0 fulls 256][kx1 halves 128][kx0 halves 128]
#   [kx2 fulls 256][kx2 halves 128][heads 40]  -> 1192 cols
FULLS = {1: 0, 0: 256, 2: 768}     # col base of the 2x128 full blocks per kx
HALVES = {1: 512, 0: 640, 2: 1024} # col base of the 2x64 half blocks per kx
WHD0 = 1152
WALLC = 1192

_NC_CACHE = {}


def _build_nc():
    from contextlib import ExitStack

    import concourse.mybir as mybir
    import concourse.tile as tile
    from concourse import bacc

    f32 = mybir.dt.float32
    bf16 = mybir.dt.bfloat16
    f8e3 = mybir.dt.float8e3
    Sigmoid = mybir.ActivationFunctionType.Sigmoid

    nc = bacc.Bacc("TRN2", target_bir_lowering=False, debug=False, num_devices=8)
    x_d = nc.dram_tensor("x", [128, HALO * W], f8e3, kind="ExternalInput").ap()
    wall_d = nc.dram_tensor("wall", [128, WALLC], bf16, kind="ExternalInput").ap()
    b_d = nc.dram_tensor("b40", [40, 1], f32, kind="ExternalInput").ap()
    out_d = nc.dram_tensor("out", [20, HS * W], f32, kind="ExternalOutput").ap()

    with ExitStack() as ctx:
        tc = ctx.enter_context(tile.TileContext(nc))
        wpool = ctx.enter_context(tc.tile_pool(name="w", bufs=1))
        bfpool = ctx.enter_context(tc.tile_pool(name="xbf", bufs=6))
        xrpool = ctx.enter_context(tc.tile_pool(name="xr", bufs=4))
        opool = ctx.enter_context(tc.tile_pool(name="ot", bufs=3))
        ppool = ctx.enter_context(tc.tile_pool(name="pp", bufs=5, space="PSUM"))
        hpool = ctx.enter_context(tc.tile_pool(name="hp", bufs=2, space="PSUM"))
        wupool = ctx.enter_context(tc.tile_pool(name="wu", bufs=1, space="PSUM"))

        # HAM warmup: dependency-free dummy matmuls issued while the weight/x
        # DMAs are in flight, so the PE clock gate opens before real work
        dum = wpool.tile([128, 64], bf16)
        nc.gpsimd.memset(dum[:], 0.0)
        dP = wupool.tile([64, 64], f32)
        for _ in range(WARM_MM):
            nc.tensor.matmul(dP[:], dum[:], dum[:], start=True, stop=True)

        # weights split across the scalar and gpsimd queues (sync carries the
        # x chunks) in group-0 use order: kx1 fulls go first on scalar to
        # unblock the first LDWEIGHTS; kx0 fulls and kx2+heads stream on the
        # otherwise-idle gpsimd queue in parallel with the halves on scalar
        wall = wpool.tile([128, WALLC], bf16)
        nc.scalar.dma_start(wall[:, 0:256], wall_d[:, 0:256])
        nc.gpsimd.dma_start(wall[:, 256:512], wall_d[:, 256:512])
        nc.scalar.dma_start(wall[:, 512:768], wall_d[:, 512:768])
        nc.gpsimd.dma_start(wall[:, 768:WALLC], wall_d[:, 768:WALLC])
        whd = wall[:, WHD0 : WHD0 + 40]
        bt2 = wpool.tile([104, 1], f32)
        nc.scalar.dma_start(bt2[0:40, :], b_d[:])
        nc.scalar.dma_start(bt2[64:104, :], b_d[:])

        chunks = [None] * NCHUNK

        def load_chunk(c):
            r0 = c * CH
            rows = min(CH, HALO - r0)
            n = rows * W
            xb = bfpool.tile([128, CH * W], f8e3, tag="xb")
            if c == 0:
                # group 0's kx-major order consumes rows 0..5 within six
                # slots; three ascending DMAs land them just in time
                nc.sync.dma_start(xb[:, 0 : 3 * W], x_d[:, 0 : 3 * W])
                nc.sync.dma_start(xb[:, 3 * W : 6 * W], x_d[:, 3 * W : 6 * W])
                nc.sync.dma_start(xb[:, 6 * W : n], x_d[:, 6 * W : n])
            else:
                nc.sync.dma_start(xb[:, 0:n], x_d[:, r0 * W : r0 * W + n])
            chunks[c] = xb

        # per-tap column windows: out[:, so0:so1] += W_kx^T @ in[:, si0:si1]
        CUTS = {0: (0, 511, 1, 512), 1: (0, 512, 0, 512), 2: (1, 512, 0, 511)}

        def row_slice(j, si0, si1):
            t = chunks[j // CH]
            o = (j % CH) * W
            return t[:, o + si0 : o + si1]

        loaded = 0

        # gpsimd sits out the last 12 pairs so its expensive (~4us) SWDGE
        # teardown drain overlaps the matmul stream instead of trailing the
        # kernel
        def flushq(p_):
            if p_ >= HS // 2 - 12:
                return (nc.sync, nc.scalar)[p_ % 2]
            return (nc.gpsimd, nc.sync, nc.scalar)[p_ % 3]

        def emit_dual_head(xr0, p0, xr1, p1):
            # head-output partition layout (channel-row-interleaved):
            #   base+2q+e   = head channel q (cls/box/dir), row-parity e
            #   base+32+2s+e = scr channel s (sigmoid; base+32 is 32-aligned
            #   for ACT), so S[base:base+40] maps 1:1 onto the pair's
            #   [20, 2, 512] output block -> single-descriptor flush
            S = opool.tile([128, 2 * W], f32, tag="S", name="S")
            hP = hpool.tile([128, W], f32, tag="hp")
            nc.tensor.matmul(hP[0:40, :], whd[:], xr0[:], start=True, stop=True)
            nc.tensor.matmul(hP[64:104, :], whd[:], xr1[:], start=True, stop=True)
            for base, col, p_ in ((0, 0, p0), (64, W, p1)):
                cs = slice(col, col + W)
                nc.vector.tensor_scalar_add(
                    S[base : base + 32, cs], hP[base : base + 32, :], bt2[base : base + 32, :]
                )
                nc.scalar.activation(
                    S[base + 32 : base + 40, cs],
                    hP[base + 32 : base + 40, :],
                    Sigmoid,
                    bias=bt2[base + 32 : base + 40, :],
                )
                flushq(p_).dma_start(
                    out_d[:, 2 * p_ * W : (2 * p_ + 2) * W], S[base : base + 40, cs]
                )

        def conv_fulls_kx(p, P, kx, first):
            # each logical full matmul = two concurrent col-tiled M=64 twins
            # sharing one moving stream, so every slot in the kernel runs in
            # (128,64) tile mode and the array never pays a mode-switch drain
            b, c = 2 * p + 1, 2 * p + 2
            si0, si1, so0, so1 = CUTS[kx]
            for t_idx, j in ((0, b), (1, c)):
                w = wall[:, FULLS[kx] + t_idx * 128 : FULLS[kx] + (t_idx + 1) * 128]
                rs = row_slice(j, si0, si1)
                nc.tensor.matmul(
                    P[0:64, so0:so1], w[:, 0:64], rs, start=first, stop=False
                )
                nc.tensor.matmul(
                    P[64:128, so0:so1], w[:, 64:128], rs, start=first, stop=False
                )
                first = False

        def conv_halves_kx(p, P, kx):
            a, d = 2 * p, 2 * p + 3
            si0, si1, so0, so1 = CUTS[kx]
            last = kx == 2
            hb = HALVES[kx]
            nc.tensor.matmul(
                P[0:64, so0:so1],
                wall[:, hb : hb + 64],
                row_slice(a, si0, si1),
                start=False,
                stop=last,
            )
            nc.tensor.matmul(
                P[64:128, so0:so1],
                wall[:, hb + 64 : hb + 128],
                row_slice(d, si0, si1),
                start=False,
                stop=last,
            )

        pend = []  # [(xr, pair_idx), ...] heads not yet emitted
        for q in range(HS // 4):  # 2-pair groups
            p0, p1 = 2 * q, 2 * q + 1
            cneed = (2 * p1 + 3) // CH
            while loaded <= min(cneed + 3, NCHUNK - 1):
                load_chunk(loaded)
                loaded += 1

            P0 = ppool.tile([128, W], f32, tag="pp")
            P1 = ppool.tile([128, W], f32, tag="pp")
            if q == 0:
                # group 0 consumes weights in descriptor-arrival order:
                # kx1 fulls, kx0 fulls, halves, then kx2 (gpsimd queue)
                for kx in (1, 0):
                    conv_fulls_kx(p0, P0, kx, first=kx == 1)
                    conv_fulls_kx(p1, P1, kx, first=kx == 1)
                for kx in (1, 0):
                    conv_halves_kx(p0, P0, kx)
                    conv_halves_kx(p1, P1, kx)
                conv_fulls_kx(p0, P0, 2, first=False)
                conv_fulls_kx(p1, P1, 2, first=False)
                conv_halves_kx(p0, P0, 2)
                xr0 = xrpool.tile([128, W], bf16, tag="xr")
                nc.vector.tensor_scalar_max(xr0[:], P0[:], 0.0)
                conv_halves_kx(p1, P1, 2)
                xr1 = xrpool.tile([128, W], bf16, tag="xr")
                nc.vector.tensor_scalar_max(xr1[:], P1[:], 0.0)
            else:
                for kx in (1, 0, 2):
                    conv_fulls_kx(p0, P0, kx, first=kx == 1)
                for kx in (1, 0, 2):
                    conv_fulls_kx(p1, P1, kx, first=kx == 1)
                for kx in (1, 0, 2):
                    conv_halves_kx(p0, P0, kx)
                xr0 = xrpool.tile([128, W], bf16, tag="xr")
                nc.vector.tensor_scalar_max(xr0[:], P0[:], 0.0)
                for kx in (1, 0, 2):
                    conv_halves_kx(p1, P1, kx)
                xr1 = xrpool.tile([128, W], bf16, tag="xr")
                nc.vector.tensor_scalar_max(xr1[:], P1[:], 0.0)

            if pend:
                emit_dual_head(pend[0][0], pend[0][1], pend[1][0], pend[1][1])
            pend = [(xr0, p0), (xr1, p1)]

        emit_dual_head(pend[0][0], pend[0][1], pend[1][0], pend[1][1])

    nc.compile()
    return nc


def _get_nc():
    if "nc" not in _NC_CACHE:
        _NC_CACHE["nc"] = _build_nc()
    return _NC_CACHE["nc"]


def _pack_weights(w_shared, w_cls, b_cls, w_box, b_box, w_dir, b_dir, w_scr, b_scr):
    # fold the fp8 x pre-scale into the conv weights
    Wt = (np.ascontiguousarray(w_shared, np.float32) / XSCALE).transpose(1, 0, 2, 3)
    wall = np.zeros((128, WALLC), np.float32)
    for kx in range(3):
        f0 = FULLS[kx]
        wall[:, f0 + 0 : f0 + 64] = Wt[:, :, 1, kx]
        wall[:, f0 + 64 : f0 + 128] = Wt[:, :, 0, kx]
        wall[:, f0 + 128 : f0 + 192] = Wt[:, :, 2, kx]
        wall[:, f0 + 192 : f0 + 256] = Wt[:, :, 1, kx]
        h0 = HALVES[kx]
        wall[:, h0 + 0 : h0 + 64] = Wt[:, :, 0, kx]
        wall[:, h0 + 64 : h0 + 128] = Wt[:, :, 2, kx]

    Wh = np.concatenate([w_cls, w_box, w_dir, w_scr], 0)[:, :, 0, 0].astype(np.float32)  # [20,64]
    bh = np.concatenate([b_cls, b_box, b_dir, b_scr], 0).astype(np.float32)  # [20]
    # channel-row-interleaved head layout: partition 2q+e = channel q of
    # row-parity e (k-half e holds that row's 64 conv channels); scr channels
    # at 32+2s+e so the sigmoid rows start 32-aligned for ACT
    for qq in range(16):
        wall[0:64, WHD0 + 2 * qq] = Wh[qq]
        wall[64:128, WHD0 + 2 * qq + 1] = Wh[qq]
    for s in range(4):
        wall[0:64, WHD0 + 32 + 2 * s] = Wh[16 + s]
        wall[64:128, WHD0 + 33 + 2 * s] = Wh[16 + s]
    import ml_dtypes

    wall = np.ascontiguousarray(wall).astype(ml_dtypes.bfloat16)
    b40 = np.empty((40,), np.float32)
    b40[0:32] = np.repeat(bh[0:16], 2)
    b40[32:40] = np.repeat(bh[16:20], 2)
    b40 = np.ascontiguousarray(b40[:, None])  # [40,1]
    return wall, b40


def _make_in_maps(inputs):
    import ml_dtypes

    feature = np.ascontiguousarray(inputs["feature"], np.float32)  # [4,128,512,512]
    B, Cin, H, Wd = feature.shape
    assert (B, Cin, H, Wd) == (4, 128, 512, 512)

    wall, b40 = _pack_weights(
        np.asarray(inputs["w_shared"]),
        np.asarray(inputs["w_cls"]), np.asarray(inputs["b_cls"]),
        np.asarray(inputs["w_box"]), np.asarray(inputs["b_box"]),
        np.asarray(inputs["w_dir"]), np.asarray(inputs["b_dir"]),
        np.asarray(inputs["w_scr"]), np.asarray(inputs["b_scr"]),
    )

    f8 = np.clip(feature * XSCALE, -15.5, 15.5).astype(ml_dtypes.float8_e3m4)

    in_maps = []
    for core in range(8):
        bi, half = core // 2, core % 2
        r0 = half * HS
        xs = np.zeros((128, HALO, W), ml_dtypes.float8_e3m4)
        lo, hi = r0 - 1, r0 + HS + 1
        slo, shi = max(lo, 0), min(hi, H)
        xs[:, slo - lo : HALO - (hi - shi), :] = f8[bi, :, slo:shi, :]
        in_maps.append(
            {
                "x": xs.reshape(128, HALO * W),
                "wall": wall,
                "b40": b40,
            }
        )
    return in_maps


def _gather(res):
    out = np.empty((4, 20, 512, 512), np.float32)
    for core in range(8):
        bi, half = core // 2, core % 2
        out[bi, :, half * HS : (half + 1) * HS, :] = res.results[core]["out"].reshape(
            20, HS, W
        )
    return out


def kernel(**inputs):
    from concourse.bass_utils import run_bass_kernel_spmd

    in_maps = _make_in_maps(inputs)
    nc = _get_nc()
    res = run_bass_kernel_spmd(nc, in_maps, core_ids=list(range(8)))
    return _gather(res)


def run_traced(**inputs):
    """Like kernel(), but returns (out, BassKernelResults) with a profile trace."""
    from concourse.bass_utils import run_bass_kernel_spmd

    in_maps = _make_in_maps(inputs)
    nc = _get_nc()
    res = run_bass_kernel_spmd(nc, in_maps, core_ids=list(range(8)), trace=True)
    return _gather(res), res


# revision 38
# speedup vs baseline: 1.0012x; 1.0012x over previous
"""Trainium2 Bass kernel for DetCenterDense: shared 3x3 conv + ReLU + four
1x1 head convs (cls/box/dir/scr, sigmoid on scr), channel-concatenated output.

Full inputs in / full output out. Sharding: 8 cores = batch(4) x H-halves(2);
each core computes a [20, 256, 512] output shard from a [128, 258, 512]
haloed input shard. No inter-core communication.

Compute structure (per core): the 3x3 conv is 9 shifted 1x1 convs accumulated
in PSUM, processing output rows in pairs packed into one PSUM tile [128, 512]
(partitions 0:64 = row y, 64:128 = row y+1). Every matmul in the kernel is an
M=64 column-tile so the PE array stays in (128,64) mode throughout (no
mode-switch drains) and independent M=64 matmuls run two-per-slot:

  - "full" taps (middle input rows, packed [W_a|W_b] weights) issue as two
    concurrent col-tiled twins sharing one moving stream;
  - "edge" taps (rows a/d, M=64 each) pair up in one slot;
  - two consecutive pairs' head matmuls (M=40) share one slot.

That is 9.5 tensor-engine slots per row pair -- the matmul-count floor for
K=128/M<=128 -- and the measured steady state is ~216 ns/slot (N=512 bf16
issue-gap at 2.4 GHz) with <0.2% stall.

Activations stream as fp8e3 (E3M4; x pre-scaled by 2 with 1/2 folded into the
bf16 conv weights) to halve SBUF read traffic -- without this, concurrent
DVE/ACT/DMA traffic degrades the matmul issue rate ~20%. Conv weights stay
bf16 (fp8 weights would breach the 2e-2 accuracy gate; rel l2 is ~1.35e-2).
ReLU runs on DVE; head bias on DVE (identity rows) and ACT (sigmoid rows).

Prologue/epilogue engineering (the stream itself is at the slot floor):
  - all conv/head weights ship as one bf16 tensor whose columns are laid
    out in first-use order and loaded as four descriptors split across the
    scalar and gpsimd queues, each landing just before its taps run; group
    0 consumes taps in descriptor-arrival order (kx1 fulls, kx0 fulls,
    halves, kx2) over a three-piece leading x chunk;
  - a burst of dependency-free N=64 dummy matmuls at program start keeps
    the PE busy through the weight/input DMA wait, releasing the HAM clock
    gate (K=4/8 -> 8/8) before the first real matmul;
  - head outputs are written channel-row-interleaved (partition 2q+e = head
    channel q of row-parity e, scr at 32+2s+e) so each row pair's whole
    [20ch x 2rows x 512] output shard flushes with ONE DMA descriptor,
    issued immediately per pair on a rotating queue -- no end-of-kernel
    descriptor backlog; gpsimd sits out the last 12 pairs so its ~4us
    SWDGE teardown drain overlaps the matmul stream.
"""

import numpy as np

HS = 256          # output rows per core shard
HALO = HS + 2     # input rows per core shard (1-row halo each side)
W = 512
CH = 8            # input rows per DMA chunk
NCHUNK = (HALO + CH - 1) // CH
XSCALE = 2.0      # fp8e3 pre-scale of x; 1/XSCALE folded into w_shared
WARM_MM = 66      # dummy matmuls to release the HAM clock gate at startup

# all conv/head weights ship as ONE bf16 dram tensor, column layout in
# first-use order so per-group DMA descriptors stream in just ahead of the
# matmuls that need them (group 0 runs fulls kx1, fulls kx0, halves, kx2):
#   [kx1 fulls 256][kx0 fulls 256][kx1 halves 128][kx0 halves 128]
#   [kx2 fulls 256][kx2 halves 128][heads 40]  -> 1192 cols
FULLS = {1: 0, 0: 256, 2: 768}     # col base of the 2x128 full blocks per kx
HALVES = {1: 512, 0: 640, 2: 1024} # col base of the 2x64 half blocks per kx
WHD0 = 1152
WALLC = 1192

_NC_CACHE = {}


def _build_nc():
    from contextlib import ExitStack

    import concourse.mybir as mybir
    import concourse.tile as tile
    from concourse import bacc

    f32 = mybir.dt.float32
    bf16 = mybir.dt.bfloat16
    f8e3 = mybir.dt.float8e3
    Sigmoid = mybir.ActivationFunctionType.Sigmoid

    nc = bacc.Bacc("TRN2", target_bir_lowering=False, debug=False, num_devices=8)
    x_d = nc.dram_tensor("x", [128, HALO * W], f8e3, kind="ExternalInput").ap()
    wall_d = nc.dram_tensor("wall", [128, WALLC], bf16, kind="ExternalInput").ap()
    b_d = nc.dram_tensor("b40", [40, 1], f32, kind="ExternalInput").ap()
    out_d = nc.dram_tensor("out", [20, HS * W], f32, kind="ExternalOutput").ap()

    with ExitStack() as ctx:
        tc = ctx.enter_context(tile.TileContext(nc))
        wpool = ctx.enter_context(tc.tile_pool(name="w", bufs=1))
        bfpool = ctx.enter_context(tc.tile_pool(name="xbf", bufs=6))
        xrpool = ctx.enter_context(tc.tile_pool(name="xr", bufs=4))
        opool = ctx.enter_context(tc.tile_pool(name="ot", bufs=3))
        ppool = ctx.enter_context(tc.tile_pool(name="pp", bufs=5, space="PSUM"))
        hpool = ctx.enter_context(tc.tile_pool(name="hp", bufs=2, space="PSUM"))
        wupool = ctx.enter_context(tc.tile_pool(name="wu", bufs=1, space="PSUM"))

        # HAM warmup: dependency-free dummy matmuls issued while the weight/x
        # DMAs are in flight, so the PE clock gate opens before real work
        dum = wpool.tile([128, 64], bf16)
        nc.gpsimd.memset(dum[:], 0.0)
        dP = wupool.tile([64, 64], f32)
        for _ in range(WARM_MM):
            nc.tensor.matmul(dP[:], dum[:], dum[:], start=True, stop=True)

        # weights split across the scalar and gpsimd queues (sync carries the
        # x chunks) in group-0 use order: kx1 fulls go first on scalar to
        # unblock the first LDWEIGHTS; kx0 fulls and kx2+heads stream on the
        # otherwise-idle gpsimd queue in parallel with the halves on scalar
        wall = wpool.tile([128, WALLC], bf16)
        nc.scalar.dma_start(wall[:, 0:256], wall_d[:, 0:256])
        nc.gpsimd.dma_start(wall[:, 256:512], wall_d[:, 256:512])
        nc.scalar.dma_start(wall[:, 512:768], wall_d[:, 512:768])
        nc.gpsimd.dma_start(wall[:, 768:WALLC], wall_d[:, 768:WALLC])
        whd = wall[:, WHD0 : WHD0 + 40]
        bt2 = wpool.tile([104, 1], f32)
        nc.scalar.dma_start(bt2[0:40, :], b_d[:])
        nc.scalar.dma_start(bt2[64:104, :], b_d[:])

        chunks = [None] * NCHUNK

        def load_chunk(c):
            r0 = c * CH
            rows = min(CH, HALO - r0)
            n = rows * W
            xb = bfpool.tile([128, CH * W], f8e3, tag="xb")
            if c == 0:
                # group 0's kx-major order consumes rows 0..5 within six
                # slots; three ascending DMAs land them just in time
                nc.sync.dma_start(xb[:, 0 : 3 * W], x_d[:, 0 : 3 * W])
                nc.sync.dma_start(xb[:, 3 * W : 6 * W], x_d[:, 3 * W : 6 * W])
                nc.sync.dma_start(xb[:, 6 * W : n], x_d[:, 6 * W : n])
            else:
                nc.sync.dma_start(xb[:, 0:n], x_d[:, r0 * W : r0 * W + n])
            chunks[c] = xb

        # per-tap column windows: out[:, so0:so1] += W_kx^T @ in[:, si0:si1]
        CUTS = {0: (0, 511, 1, 512), 1: (0, 512, 0, 512), 2: (1, 512, 0, 511)}

        def row_slice(j, si0, si1):
            t = chunks[j // CH]
            o = (j % CH) * W
            return t[:, o + si0 : o + si1]

        loaded = 0

        # gpsimd sits out the last 12 pairs so its expensive (~4us) SWDGE
        # teardown drain overlaps the matmul stream instead of trailing the
        # kernel
        def flushq(p_):
            if p_ >= HS // 2 - 12:
                return (nc.sync, nc.scalar)[p_ % 2]
            return (nc.gpsimd, nc.sync, nc.scalar)[p_ % 3]

        def emit_dual_head(xr0, p0, xr1, p1):
            # head-output partition layout (channel-row-interleaved):
            #   base+2q+e   = head channel q (cls/box/dir), row-parity e
            #   base+32+2s+e = scr channel s (sigmoid; base+32 is 32-aligned
            #   for ACT), so S[base:base+40] maps 1:1 onto the pair's
            #   [20, 2, 512] output block -> single-descriptor flush
            S = opool.tile([128, 2 * W], f32, tag="S", name="S")
            hP = hpool.tile([128, W], f32, tag="hp")
            nc.tensor.matmul(hP[0:40, :], whd[:], xr0[:], start=True, stop=True)
            nc.tensor.matmul(hP[64:104, :], whd[:], xr1[:], start=True, stop=True)
            for base, col, p_ in ((0, 0, p0), (64, W, p1)):
                cs = slice(col, col + W)
                nc.vector.tensor_scalar_add(
                    S[base : base + 32, cs], hP[base : base + 32, :], bt2[base : base + 32, :]
                )
                nc.scalar.activation(
                    S[base + 32 : base + 40, cs],
                    hP[base + 32 : base + 40, :],
                    Sigmoid,
                    bias=bt2[base + 32 : base + 40, :],
                )
                flushq(p_).dma_start(
                    out_d[:, 2 * p_ * W : (2 * p_ + 2) * W], S[base : base + 40, cs]
                )

        def conv_fulls_kx(p, P, kx, first):
            # each logical full matmul = two concurrent col-tiled M=64 twins
            # sharing one moving stream, so every slot in the kernel runs in
            # (128,64) tile mode and the array never pays a mode-switch drain
            b, c = 2 * p + 1, 2 * p + 2
            si0, si1, so0, so1 = CUTS[kx]
            for t_idx, j in ((0, b), (1, c)):
                w = wall[:, FULLS[kx] + t_idx * 128 : FULLS[kx] + (t_idx + 1) * 128]
                rs = row_slice(j, si0, si1)
                nc.tensor.matmul(
                    P[0:64, so0:so1], w[:, 0:64], rs, start=first, stop=False
                )
                nc.tensor.matmul(
                    P[64:128, so0:so1], w[:, 64:128], rs, start=first, stop=False
                )
                first = False

        def conv_halves_kx(p, P, kx):
            a, d = 2 * p, 2 * p + 3
            si0, si1, so0, so1 = CUTS[kx]
            last = kx == 2
            hb = HALVES[kx]
            nc.tensor.matmul(
                P[0:64, so0:so1],
                wall[:, hb : hb + 64],
                row_slice(a, si0, si1),
                start=False,
                stop=last,
            )
            nc.tensor.matmul(
                P[64:128, so0:so1],
                wall[:, hb + 64 : hb + 128],
                row_slice(d, si0, si1),
                start=False,
                stop=last,
            )

        pend = []  # [(xr, pair_idx), ...] heads not yet emitted
        for q in range(HS // 4):  # 2-pair groups
            p0, p1 = 2 * q, 2 * q + 1
            cneed = (2 * p1 + 3) // CH
            while loaded <= min(cneed + 3, NCHUNK - 1):
                load_chunk(loaded)
                loaded += 1

            if pend and q == HS // 4 - 1:
                # last group only: emit the pending dual-head up front. The
                # head matmul pays one ~0.5us stall on the previous ReLU,
                # but its add/sigmoid/flush chain then overlaps this whole
                # group instead of serializing after the final matmul
                # (doing this every group costs ~0.5us x 64 - only the last
                # group's tail is worth buying back)
                emit_dual_head(pend[0][0], pend[0][1], pend[1][0], pend[1][1])
                pend = []

            P0 = ppool.tile([128, W], f32, tag="pp")
            P1 = ppool.tile([128, W], f32, tag="pp")
            if q == 0:
                # group 0 consumes weights in descriptor-arrival order:
                # kx1 fulls, kx0 fulls, halves, then kx2 (gpsimd queue)
                for kx in (1, 0):
                    conv_fulls_kx(p0, P0, kx, first=kx == 1)
                    conv_fulls_kx(p1, P1, kx, first=kx == 1)
                for kx in (1, 0):
                    conv_halves_kx(p0, P0, kx)
                    conv_halves_kx(p1, P1, kx)
                conv_fulls_kx(p0, P0, 2, first=False)
                conv_fulls_kx(p1, P1, 2, first=False)
                conv_halves_kx(p0, P0, 2)
                xr0 = xrpool.tile([128, W], bf16, tag="xr")
                nc.vector.tensor_scalar_max(xr0[:], P0[:], 0.0)
                conv_halves_kx(p1, P1, 2)
                xr1 = xrpool.tile([128, W], bf16, tag="xr")
                nc.vector.tensor_scalar_max(xr1[:], P1[:], 0.0)
            else:
                for kx in (1, 0, 2):
                    conv_fulls_kx(p0, P0, kx, first=kx == 1)
                for kx in (1, 0, 2):
                    conv_fulls_kx(p1, P1, kx, first=kx == 1)
                for kx in (1, 0, 2):
                    conv_halves_kx(p0, P0, kx)
                xr0 = xrpool.tile([128, W], bf16, tag="xr")
                nc.vector.tensor_scalar_max(xr0[:], P0[:], 0.0)
                for kx in (1, 0, 2):
                    conv_halves_kx(p1, P1, kx)
                xr1 = xrpool.tile([128, W], bf16, tag="xr")
                nc.vector.tensor_scalar_max(xr1[:], P1[:], 0.0)

            if pend:
                emit_dual_head(pend[0][0], pend[0][1], pend[1][0], pend[1][1])
            pend = [(xr0, p0), (xr1, p1)]

        emit_dual_head(pend[0][0], pend[0][1], pend[1][0], pend[1][1])

    nc.compile()
    return nc


def _get_nc():
    if "nc" not in _NC_CACHE:
        _NC_CACHE["nc"] = _build_nc()
    return _NC_CACHE["nc"]


def _pack_weights(w_shared, w_cls, b_cls, w_box, b_box, w_dir, b_dir, w_scr, b_scr):
    # fold the fp8 x pre-scale into the conv weights
    Wt = (np.ascontiguousarray(w_shared, np.float32) / XSCALE).transpose(1, 0, 2, 3)
    wall = np.zeros((128, WALLC), np.float32)
    for kx in range(3):
        f0 = FULLS[kx]
        wall[:, f0 + 0 : f0 + 64] = Wt[:, :, 1, kx]
        wall[:, f0 + 64 : f0 + 128] = Wt[:, :, 0, kx]
        wall[:, f0 + 128 : f0 + 192] = Wt[:, :, 2, kx]
        wall[:, f0 + 192 : f0 + 256] = Wt[:, :, 1, kx]
        h0 = HALVES[kx]
        wall[:, h0 + 0 : h0 + 64] = Wt[:, :, 0, kx]
        wall[:, h0 + 64 : h0 + 128] = Wt[:, :, 2, kx]

    Wh = np.concatenate([w_cls, w_box, w_dir, w_scr], 0)[:, :, 0, 0].astype(np.float32)  # [20,64]
    bh = np.concatenate([b_cls, b_box, b_dir, b_scr], 0).astype(np.float32)  # [20]
    # channel-row-interleaved head layout: partition 2q+e = channel q of
    # row-parity e (k-half e holds that row's 64 conv channels); scr channels
    # at 32+2s+e so the sigmoid rows start 32-aligned for ACT
    for qq in range(16):
        wall[0:64, WHD0 + 2 * qq] = Wh[qq]
        wall[64:128, WHD0 + 2 * qq + 1] = Wh[qq]
    for s in range(4):
        wall[0:64, WHD0 + 32 + 2 * s] = Wh[16 + s]
        wall[64:128, WHD0 + 33 + 2 * s] = Wh[16 + s]
    import ml_dtypes

    wall = np.ascontiguousarray(wall).astype(ml_dtypes.bfloat16)
    b40 = np.empty((40,), np.float32)
    b40[0:32] = np.repeat(bh[0:16], 2)
    b40[32:40] = np.repeat(bh[16:20], 2)
    b40 = np.ascontiguousarray(b40[:, None])  # [40,1]
    return wall, b40


def _make_in_maps(inputs):
    import ml_dtypes

    feature = np.ascontiguousarray(inputs["feature"], np.float32)  # [4,128,512,512]
    B, Cin, H, Wd = feature.shape
    assert (B, Cin, H, Wd) == (4, 128, 512, 512)

    wall, b40 = _pack_weights(
        np.asarray(inputs["w_shared"]),
        np.asarray(inputs["w_cls"]), np.asarray(inputs["b_cls"]),
        np.asarray(inputs["w_box"]), np.asarray(inputs["b_box"]),
        np.asarray(inputs["w_dir"]), np.asarray(inputs["b_dir"]),
        np.asarray(inputs["w_scr"]), np.asarray(inputs["b_scr"]),
    )

    f8 = np.clip(feature * XSCALE, -15.5, 15.5).astype(ml_dtypes.float8_e3m4)

    in_maps = []
    for core in range(8):
        bi, half = core // 2, core % 2
        r0 = half * HS
        xs = np.zeros((128, HALO, W), ml_dtypes.float8_e3m4)
        lo, hi = r0 - 1, r0 + HS + 1
        slo, shi = max(lo, 0), min(hi, H)
        xs[:, slo - lo : HALO - (hi - shi), :] = f8[bi, :, slo:shi, :]
        in_maps.append(
            {
                "x": xs.reshape(128, HALO * W),
                "wall": wall,
                "b40": b40,
            }
        )
    return in_maps


def _gather(res):
    out = np.empty((4, 20, 512, 512), np.float32)
    for core in range(8):
        bi, half = core // 2, core % 2
        out[bi, :, half * HS : (half + 1) * HS, :] = res.results[core]["out"].reshape(
            20, HS, W
        )
    return out


def kernel(**inputs):
    from concourse.bass_utils import run_bass_kernel_spmd

    in_maps = _make_in_maps(inputs)
    nc = _get_nc()
    res = run_bass_kernel_spmd(nc, in_maps, core_ids=list(range(8)))
    return _gather(res)


def run_traced(**inputs):
    """Like kernel(), but returns (out, BassKernelResults) with a profile trace."""
    from concourse.bass_utils import run_bass_kernel_spmd

    in_maps = _make_in_maps(inputs)
    nc = _get_nc()
    res = run_bass_kernel_spmd(nc, in_maps, core_ids=list(range(8)), trace=True)
    return _gather(res), res
